# revision 12
# baseline (speedup 1.0000x reference)
"""GCN (2x GCNConv + global_max_pool + 3-layer MLP) on 8 Trainium2 NeuronCores.

Strategy (graph/data parallel per the sharding hint):
  - Nodes are sharded across the 8 cores in contiguous, graph-aligned ranges
    (batch is sorted, so original node order == graph order).
  - The edge list is partitioned by destination node; each core aggregates
    messages for its own destination shard.
  - Aggregation kernel per core: dma_gather of source-node feature rows
    (rows padded to 128 f32 = 512B for full DMA efficiency), then a
    one-hot-weighted matmul scatter: for each 128-edge block, a narrow
    one-hot matrix S[e, dslot] = norm_e is built on the Vector engine with a
    single dual-op tensor_scalar (is_equal x mult against an iota constant),
    and PE accumulates  PSUM[feat, slot] += msg_block^T @ S  into a
    512-slot-wide PSUM supertile.  Self-loops are appended as ordinary edges.
  - Degree normalization coefficients (pure graph-structure quantities) are
    precomputed on the host as part of edge-list partitioning.
  - Layer 1 fuses the aggregation with the dense stages
    h1 = relu(agg1 @ W1 + b1) and p = h1 @ W2 entirely on-chip, streaming
    per 512-node supertile; p is written back node-sharded.
  - Between the two GCN layers the full gather table p must be visible to
    every core (edges cross shards); the shards are exchanged via the host
    (pure concatenation / layout change) between two device launches.
  - Layer 2 aggregates p, applies relu(. + b2), computes per-graph max-pool
    partials for the core's own graphs, and outputs a [115, 256] partial.
  - Host combines pooled partials with an elementwise max (the "all-reduce
    only the tiny pooled per-graph features" step of the sharding hint) and
    a final tiny launch runs the 3-layer MLP head on one core.

Each core gets its own tailor-made program (edge structure is data
dependent); the 8 programs are dispatched concurrently through the same
bass2jax/PJRT path that bass_utils.run_bass_kernel_spmd uses under axon.
"""

import sys

sys.path.insert(0, "/opt/trn_rl_repo")

import numpy as np

import concourse.bass as bass
import concourse.bacc as bacc
import concourse.tile as tile
from concourse import mybir

# ----------------------------------------------------------------------------
# Model / layout constants
# ----------------------------------------------------------------------------
N_NODES = 100000
N_GRAPHS = 256
F_IN = 114  # x features
F_H = 230  # hidden (W1 out)
F_P = 115  # W2 out == layer-2 message width
RP = 128  # padded row width of gather tables (512B)
SUPER = 512  # PSUM supertile width in node slots (one 2KB fp32 bank)
SG = 2  # supertiles per gather call-group
N_CORES = 8
N_CHUNKS = 4  # gather-table split so int16 indices suffice

DT = mybir.dt.float32
AX = mybir.AxisListType
ALU = mybir.AluOpType
ACTF = mybir.ActivationFunctionType


# ----------------------------------------------------------------------------
# Host-side preprocessing: edge partitioning / ordering / metadata
# ----------------------------------------------------------------------------
class CoreMeta:
    __slots__ = (
        "ns",
        "ne",
        "n_slots",
        "n_super",
        "calls",
        "passes",
        "idx_dram",
        "meta_dram",
        "graphs",
        "n_blocks",
    )


def _shard_boundaries(batch):
    counts = np.bincount(batch, minlength=N_GRAPHS)
    cum = np.concatenate([[0], np.cumsum(counts)])  # cum[g] = first node of graph g
    bounds = [0]
    for c in range(1, N_CORES):
        target = c * batch.shape[0] / N_CORES
        g = int(np.argmin(np.abs(cum - target)))
        g = max(g, 1)
        bounds.append(int(cum[g]))
    bounds.append(int(batch.shape[0]))
    gb = [int(np.searchsorted(cum, b)) for b in bounds]  # graph index per boundary
    return bounds, cum, gb


def preprocess(edge_index, batch, npad, chunk):
    """Build per-core edge metadata. Pure index/layout work + the standard
    GCN degree-normalization coefficients (functions of edge_index only)."""
    src = np.asarray(edge_index[0], dtype=np.int64)
    dst = np.asarray(edge_index[1], dtype=np.int64)
    batch = np.asarray(batch, dtype=np.int64)
    n = batch.shape[0]

    deg = (np.bincount(dst, minlength=n) + 1.0).astype(np.float32)
    dinv = (1.0 / np.sqrt(deg)).astype(np.float32)

    bounds, cum, gb = _shard_boundaries(batch)

    metas = []
    for c in range(N_CORES):
        ns, ne = bounds[c], bounds[c + 1]
        m = CoreMeta()
        m.ns, m.ne = ns, ne
        m.n_slots = ne - ns
        m.n_super = -(-m.n_slots // SUPER)

        emask = (dst >= ns) & (dst < ne)
        es = src[emask]
        ed = dst[emask]
        enorm = dinv[es] * dinv[ed]
        # self loops
        sn = np.arange(ns, ne, dtype=np.int64)
        es = np.concatenate([es, sn])
        ed = np.concatenate([ed, sn])
        enorm = np.concatenate([enorm, dinv[sn] * dinv[sn]]).astype(np.float32)

        dslot = ed - ns
        sup = dslot // SUPER
        sgrp = sup // SG
        ch = es // chunk
        order = np.lexsort((dslot, ch, sgrp))
        es, dslot, enorm, ch, sgrp = (
            es[order],
            dslot[order],
            enorm[order],
            ch[order],
            sgrp[order],
        )

        # pad each (sgrp, chunk) run to a multiple of 128
        run_key = sgrp * N_CHUNKS + ch
        run_starts = np.flatnonzero(np.diff(run_key, prepend=-1))
        run_ends = np.append(run_starts[1:], run_key.shape[0])

        idx_parts = []  # int16 gather indices per run
        dsl_parts = []  # per-edge slot (>=0) or -1 pad
        nrm_parts = []
        calls = []  # (chunk, sgrp, col0, n_idx, blk0)
        blk = 0
        col = 0
        for rs, re_ in zip(run_starts, run_ends):
            cnum = int(ch[rs])
            npadded = -(-(re_ - rs) // 128) * 128
            pad = npadded - (re_ - rs)
            idx = (es[rs:re_] - cnum * chunk).astype(np.int16)
            idx_parts.append(np.concatenate([idx, np.zeros(pad, np.int16)]))
            dsl_parts.append(
                np.concatenate([dslot[rs:re_], np.full(pad, -1, np.int64)])
            )
            nrm_parts.append(
                np.concatenate([enorm[rs:re_], np.zeros(pad, np.float32)])
            )
            calls.append((cnum, int(sgrp[rs]), col, npadded, blk))
            col += npadded // 16
            blk += npadded // 128

        idx_all = np.concatenate(idx_parts)
        dsl_all = np.concatenate(dsl_parts)
        nrm_all = np.concatenate(nrm_parts)
        m.n_blocks = blk

        # idx stream wrapped in 16 partitions, replicated to 128
        idx_mat = idx_all.reshape(-1, 16).T  # [16, cols]
        m.idx_dram = np.ascontiguousarray(np.tile(idx_mat, (8, 1)))  # [128, cols]

        # passes: for each 128-edge block, one pass per supertile present
        passes = []  # (blk, super, lo, W, meta_idx)
        meta_cols = []
        dsl_b = dsl_all.reshape(-1, 128)
        nrm_b = nrm_all.reshape(-1, 128)
        for b in range(blk):
            d = dsl_b[b]
            real = d >= 0
            sups = np.unique(d[real] // SUPER)
            for s in sups:
                sel = real & (d // SUPER == s)
                rel = d - s * SUPER
                lo = int(rel[sel].min())
                hi = int(rel[sel].max())
                col_d = np.where(sel, rel, -1).astype(np.float32)
                col_n = nrm_b[b].astype(np.float32)
                passes.append((b, int(s), lo, hi - lo + 1, len(meta_cols)))
                meta_cols.append((col_d, col_n))
        m.passes = passes
        m.calls = calls
        meta = np.zeros((128, 2 * max(1, len(meta_cols))), np.float32)
        for k, (cd, cn) in enumerate(meta_cols):
            meta[:, 2 * k] = cd
            meta[:, 2 * k + 1] = cn
        m.meta_dram = meta

        # graphs owned by this core: global id + column range in shard
        m.graphs = []
        for g in range(gb[c], gb[c + 1]):
            cs, ce = int(cum[g]) - ns, int(cum[g + 1]) - ns
            if ce > cs:
                m.graphs.append((g, cs, ce))
        metas.append(m)
    return metas, dinv


# ----------------------------------------------------------------------------
# Program builders
# ----------------------------------------------------------------------------
def _iota_const():
    a = np.tile(np.arange(SUPER, dtype=np.float32), (128, 1))
    return np.ascontiguousarray(a)


def _emit_aggregation(nc, tc, ctx, m, table_dram, idx_sb, meta_sb, iota_sb, fdim, chunk, on_super_done):
    """Emit gather + one-hot matmul aggregation. Calls on_super_done(s, psum)
    after each supertile's accumulation completes."""
    g_pool = ctx.enter_context(tc.tile_pool(name="gather", bufs=2 * N_CHUNKS + 2))
    oh_pool = ctx.enter_context(tc.tile_pool(name="onehot", bufs=8))
    ps_pool = ctx.enter_context(tc.tile_pool(name="psum_agg", bufs=3, space="PSUM"))

    # group calls/passes by sgroup
    n_sgrp = -(-m.n_super // SG)
    calls_by_sg = [[] for _ in range(n_sgrp)]
    for cll in m.calls:
        calls_by_sg[cll[1]].append(cll)
    passes_by_super = [[] for _ in range(m.n_super)]
    for p in m.passes:
        passes_by_super[p[1]].append(p)

    # block -> (call, rel) mapping
    blk_call = {}
    call_tiles = {}
    for k, (cnum, sg, col0, n_idx, blk0) in enumerate(m.calls):
        for b in range(n_idx // 128):
            blk_call[blk0 + b] = (k, b)

    for sg in range(n_sgrp):
        for k, cll in enumerate(calls_by_sg[sg]):
            cnum, _, col0, n_idx, blk0 = cll
            nb = n_idx // 128
            gt = g_pool.tile([128, nb, RP], DT, tag="gather")
            nc.gpsimd.dma_gather(
                out_ap=gt[:, :, :],
                in_ap=table_dram[cnum * chunk : (cnum + 1) * chunk, :],
                idxs_ap=idx_sb[:, col0 : col0 + n_idx // 16],
                num_idxs=n_idx,
                num_idxs_reg=n_idx,
                elem_size=RP,
                elem_step=RP,
                single_packet=False,
            )
            ci = m.calls.index(cll)
            call_tiles[ci] = gt

        for s in range(sg * SG, min((sg + 1) * SG, m.n_super)):
            psum = ps_pool.tile([128, SUPER], DT, tag="agg")
            nc.vector.memset(psum[:fdim, :], 0.0)
            for (b, _s, lo, w, mi) in passes_by_super[s]:
                ck, brel = blk_call[b]
                gt = call_tiles[ck]
                oh = oh_pool.tile([128, 128], DT, tag="oh")
                wv = min(w, 128)
                nc.vector.tensor_scalar(
                    out=oh[:, :wv],
                    in0=iota_sb[:, lo : lo + wv],
                    scalar1=meta_sb[:, 2 * mi : 2 * mi + 1],
                    scalar2=meta_sb[:, 2 * mi + 1 : 2 * mi + 2],
                    op0=ALU.is_equal,
                    op1=ALU.mult,
                )
                if w > 128:  # rare wide window: second strip
                    oh2 = oh_pool.tile([128, 128], DT, tag="oh")
                    w2 = w - 128
                    nc.vector.tensor_scalar(
                        out=oh2[:, :w2],
                        in0=iota_sb[:, lo + 128 : lo + w],
                        scalar1=meta_sb[:, 2 * mi : 2 * mi + 1],
                        scalar2=meta_sb[:, 2 * mi + 1 : 2 * mi + 2],
                        op0=ALU.is_equal,
                        op1=ALU.mult,
                    )
                    nc.tensor.matmul(
                        out=psum[:fdim, lo + 128 : lo + w],
                        lhsT=gt[:, brel, :fdim],
                        rhs=oh2[:, :w2],
                        start=False,
                        stop=False,
                        skip_group_check=True,
                    )
                nc.tensor.matmul(
                    out=psum[:fdim, lo : lo + wv],
                    lhsT=gt[:, brel, :fdim],
                    rhs=oh[:, :wv],
                    start=False,
                    stop=False,
                    skip_group_check=True,
                )
            on_super_done(s, psum)


def build_launch1(m, chunk, npad):
    nc = bacc.Bacc("TRN2", target_bir_lowering=False, debug=False, num_devices=1)
    x_pad = nc.dram_tensor("x_pad", [npad, RP], DT, kind="ExternalInput").ap()
    idx_d = nc.dram_tensor("idx", list(m.idx_dram.shape), mybir.dt.int16, kind="ExternalInput").ap()
    meta_d = nc.dram_tensor("meta", list(m.meta_dram.shape), DT, kind="ExternalInput").ap()
    iota_d = nc.dram_tensor("iota", [128, SUPER], DT, kind="ExternalInput").ap()
    w1_d = nc.dram_tensor("W1", [F_IN, F_H], DT, kind="ExternalInput").ap()
    w2_d = nc.dram_tensor("W2", [F_H, F_P], DT, kind="ExternalInput").ap()
    b1_d = nc.dram_tensor("b1", [F_H, 1], DT, kind="ExternalInput").ap()
    ncol = m.n_super * SUPER
    pt_d = nc.dram_tensor("pT", [F_P, ncol], DT, kind="ExternalOutput").ap()

    from contextlib import ExitStack

    with tile.TileContext(nc) as tc:
        with ExitStack() as ctx:
            cpool = ctx.enter_context(tc.tile_pool(name="consts", bufs=1))
            idx_sb = cpool.tile([128, m.idx_dram.shape[1]], mybir.dt.int16)
            nc.sync.dma_start(idx_sb[:, :], idx_d[:, :])
            meta_sb = cpool.tile([128, m.meta_dram.shape[1]], DT)
            nc.sync.dma_start(meta_sb[:, :], meta_d[:, :])
            iota_sb = cpool.tile([128, SUPER], DT)
            nc.sync.dma_start(iota_sb[:, :], iota_d[:, :])
            w1a = cpool.tile([F_IN, F_P], DT)
            nc.sync.dma_start(w1a[:, :], w1_d[:, 0:F_P])
            w1b = cpool.tile([F_IN, F_H - F_P], DT)
            nc.sync.dma_start(w1b[:, :], w1_d[:, F_P:F_H])
            w2a = cpool.tile([F_P, F_P], DT)
            nc.sync.dma_start(w2a[:, :], w2_d[0:F_P, :])
            w2b = cpool.tile([F_H - F_P, F_P], DT)
            nc.sync.dma_start(w2b[:, :], w2_d[F_P:F_H, :])
            b1a = cpool.tile([F_P, 1], DT)
            nc.sync.dma_start(b1a[:, :], b1_d[0:F_P, :])
            b1b = cpool.tile([F_H - F_P, 1], DT)
            nc.sync.dma_start(b1b[:, :], b1_d[F_P:F_H, :])

            sb_pool = ctx.enter_context(tc.tile_pool(name="dense_sb", bufs=3))
            psd_pool = ctx.enter_context(tc.tile_pool(name="psum_dense", bufs=2, space="PSUM"))

            def on_super(s, psum):
                # evacuate aggregation
                agg = sb_pool.tile([F_IN, SUPER], DT, tag="agg_sb")
                nc.scalar.copy(agg[:, :], psum[:F_IN, :])
                # h1 = relu(W1^T agg + b1)  (two column halves)
                h1ps_a = psd_pool.tile([128, SUPER], DT, tag="h1ps")
                nc.tensor.matmul(out=h1ps_a[:F_P, :], lhsT=w1a[:, :], rhs=agg[:, :], start=True, stop=True)
                h1ps_b = psd_pool.tile([128, SUPER], DT, tag="h1ps")
                nc.tensor.matmul(out=h1ps_b[: F_H - F_P, :], lhsT=w1b[:, :], rhs=agg[:, :], start=True, stop=True)
                h1a = sb_pool.tile([F_P, SUPER], DT, tag="h1a")
                nc.scalar.activation(h1a[:, :], h1ps_a[:F_P, :], ACTF.Relu, bias=b1a[:, 0:1])
                h1b = sb_pool.tile([F_H - F_P, SUPER], DT, tag="h1b")
                nc.scalar.activation(h1b[:, :], h1ps_b[: F_H - F_P, :], ACTF.Relu, bias=b1b[:, 0:1])
                # p = W2^T h1  (contraction split over the two halves)
                pps = psd_pool.tile([128, SUPER], DT, tag="pps")
                nc.tensor.matmul(out=pps[:F_P, :], lhsT=w2a[:, :], rhs=h1a[:, :], start=True, stop=False)
                nc.tensor.matmul(out=pps[:F_P, :], lhsT=w2b[:, :], rhs=h1b[:, :], start=False, stop=True)
                psb = sb_pool.tile([F_P, SUPER], DT, tag="psb")
                nc.scalar.copy(psb[:, :], pps[:F_P, :])
                nc.sync.dma_start(pt_d[:, s * SUPER : (s + 1) * SUPER], psb[:, :])

            _emit_aggregation(nc, tc, ctx, m, x_pad, idx_sb, meta_sb, iota_sb, F_IN, chunk, on_super)

    nc.compile()
    return nc


def build_launch2(m, chunk, npad):
    nc = bacc.Bacc("TRN2", target_bir_lowering=False, debug=False, num_devices=1)
    p_pad = nc.dram_tensor("p_pad", [npad, RP], DT, kind="ExternalInput").ap()
    idx_d = nc.dram_tensor("idx", list(m.idx_dram.shape), mybir.dt.int16, kind="ExternalInput").ap()
    meta_d = nc.dram_tensor("meta", list(m.meta_dram.shape), DT, kind="ExternalInput").ap()
    iota_d = nc.dram_tensor("iota", [128, SUPER], DT, kind="ExternalInput").ap()
    b2_d = nc.dram_tensor("b2", [F_P, 1], DT, kind="ExternalInput").ap()
    pooled_d = nc.dram_tensor("pooled", [F_P, N_GRAPHS], DT, kind="ExternalOutput").ap()

    from contextlib import ExitStack

    with tile.TileContext(nc) as tc:
        with ExitStack() as ctx:
            cpool = ctx.enter_context(tc.tile_pool(name="consts", bufs=1))
            idx_sb = cpool.tile([128, m.idx_dram.shape[1]], mybir.dt.int16)
            nc.sync.dma_start(idx_sb[:, :], idx_d[:, :])
            meta_sb = cpool.tile([128, m.meta_dram.shape[1]], DT)
            nc.sync.dma_start(meta_sb[:, :], meta_d[:, :])
            iota_sb = cpool.tile([128, SUPER], DT)
            nc.sync.dma_start(iota_sb[:, :], iota_d[:, :])
            b2 = cpool.tile([F_P, 1], DT)
            nc.sync.dma_start(b2[:, :], b2_d[:, :])
            h2 = cpool.tile([F_P, m.n_super * SUPER], DT)
            pooled = cpool.tile([F_P, N_GRAPHS], DT)
            nc.vector.memset(pooled[:, :], 0.0)

            def on_super(s, psum):
                nc.scalar.activation(
                    h2[:, s * SUPER : (s + 1) * SUPER], psum[:F_P, :], ACTF.Relu, bias=b2[:, 0:1]
                )

            _emit_aggregation(nc, tc, ctx, m, p_pad, idx_sb, meta_sb, iota_sb, F_P, chunk, on_super)

            for g, cs, ce in m.graphs:
                nc.vector.tensor_reduce(
                    out=pooled[:, g : g + 1],
                    in_=h2[:, cs:ce],
                    axis=AX.X,
                    op=ALU.max,
                )
            nc.sync.dma_start(pooled_d[:, :], pooled[:, :])

    nc.compile()
    return nc


def build_launch3():
    nc = bacc.Bacc("TRN2", target_bir_lowering=False, debug=False, num_devices=1)
    pooled_d = nc.dram_tensor("pooled", [F_P, N_GRAPHS], DT, kind="ExternalInput").ap()
    wg_d = nc.dram_tensor("Wg", [F_P, 64], DT, kind="ExternalInput").ap()
    bg_d = nc.dram_tensor("bg", [64, 1], DT, kind="ExternalInput").ap()
    wf_d = nc.dram_tensor("Wf", [64, 32], DT, kind="ExternalInput").ap()
    bf_d = nc.dram_tensor("bf", [32, 1], DT, kind="ExternalInput").ap()
    wo_d = nc.dram_tensor("Wo", [32, 1], DT, kind="ExternalInput").ap()
    bo_d = nc.dram_tensor("bo", [1, 1], DT, kind="ExternalInput").ap()
    out_d = nc.dram_tensor("out", [1, N_GRAPHS], DT, kind="ExternalOutput").ap()

    from contextlib import ExitStack

    with tile.TileContext(nc) as tc:
        with ExitStack() as ctx:
            pool = ctx.enter_context(tc.tile_pool(name="mlp", bufs=1))
            pspool = ctx.enter_context(tc.tile_pool(name="mlp_ps", bufs=1, space="PSUM"))
            pooled = pool.tile([F_P, N_GRAPHS], DT)
            nc.sync.dma_start(pooled[:, :], pooled_d[:, :])
            wg = pool.tile([F_P, 64], DT)
            nc.sync.dma_start(wg[:, :], wg_d[:, :])
            bg = pool.tile([64, 1], DT)
            nc.sync.dma_start(bg[:, :], bg_d[:, :])
            wf = pool.tile([64, 32], DT)
            nc.sync.dma_start(wf[:, :], wf_d[:, :])
            bf = pool.tile([32, 1], DT)
            nc.sync.dma_start(bf[:, :], bf_d[:, :])
            wo = pool.tile([32, 1], DT)
            nc.sync.dma_start(wo[:, :], wo_d[:, :])
            bo = pool.tile([1, 1], DT)
            nc.sync.dma_start(bo[:, :], bo_d[:, :])

            ps1 = pspool.tile([64, N_GRAPHS], DT)
            nc.tensor.matmul(out=ps1[:, :], lhsT=wg[:, :], rhs=pooled[:, :], start=True, stop=True)
            g1 = pool.tile([64, N_GRAPHS], DT)
            nc.scalar.activation(g1[:, :], ps1[:, :], ACTF.Relu, bias=bg[:, 0:1])
            ps2 = pspool.tile([32, N_GRAPHS], DT)
            nc.tensor.matmul(out=ps2[:, :], lhsT=wf[:, :], rhs=g1[:, :], start=True, stop=True)
            g2 = pool.tile([32, N_GRAPHS], DT)
            nc.scalar.activation(g2[:, :], ps2[:, :], ACTF.Relu, bias=bf[:, 0:1])
            ps3 = pspool.tile([1, N_GRAPHS], DT)
            nc.tensor.matmul(out=ps3[:, :], lhsT=wo[:, :], rhs=g2[:, :], start=True, stop=True)
            og = pool.tile([1, N_GRAPHS], DT)
            nc.vector.tensor_scalar(
                out=og[:, :], in0=ps3[:, :], scalar1=bo[:, 0:1], scalar2=None, op0=ALU.add
            )
            nc.sync.dma_start(out_d[:, :], og[:, :])

    nc.compile()
    return nc


# ----------------------------------------------------------------------------
# Concurrent multi-program PJRT runner (derived from the axon redirect path
# of bass_utils.run_bass_kernel_spmd / bass2jax.run_bass_via_pjrt).
# ----------------------------------------------------------------------------
def _introspect(nc):
    in_names, out_names, out_avals, zero_outs = [], [], [], []
    import jax

    pname = nc.partition_id_tensor.name if nc.partition_id_tensor else None
    for alloc in nc.m.functions[0].allocations:
        if not isinstance(alloc, mybir.MemoryLocationSet):
            continue
        name = alloc.memorylocations[0].name
        if alloc.kind == "ExternalInput":
            if name != pname:
                in_names.append(name)
        elif alloc.kind == "ExternalOutput":
            shape = tuple(alloc.tensor_shape)
            dtype = mybir.dt.np(alloc.dtype)
            out_names.append(name)
            out_avals.append(jax.core.ShapedArray(shape, dtype))
            zero_outs.append(np.zeros(shape, dtype))
    return in_names, out_names, out_avals, zero_outs


def run_programs(jobs, devices=None):
    """jobs: list of (nc, in_map, device_index). Returns list of out dicts.
    All programs are dispatched asynchronously and run concurrently."""
    import jax
    from concourse.bass2jax import _bass_exec_p, install_neuronx_cc_hook

    install_neuronx_cc_hook()
    if devices is None:
        devices = jax.devices()

    futures = []
    for nc, in_map, di in jobs:
        in_names, out_names, out_avals, zero_outs = _introspect(nc)
        n_params = len(in_names)
        pname = nc.partition_id_tensor.name if nc.partition_id_tensor else None
        all_names = tuple(in_names + out_names + ([pname] if pname else []))

        def make_body(nc=nc, out_avals=tuple(out_avals), all_names=all_names, out_names=tuple(out_names), pname=pname):
            def _body(*args):
                operands = list(args)
                if pname:
                    from concourse.bass2jax import partition_id_tensor

                    operands.append(partition_id_tensor())
                outs = _bass_exec_p.bind(
                    *operands,
                    out_avals=out_avals,
                    in_names=all_names,
                    out_names=out_names,
                    lowering_input_output_aliases=(),
                    sim_require_finite=False,
                    sim_require_nnan=False,
                    nc=nc,
                )
                return tuple(outs)

            return _body

        donate = tuple(range(n_params, n_params + len(out_names)))
        dev = devices[di]
        args = [jax.device_put(np.ascontiguousarray(in_map[nm]), dev) for nm in in_names]
        args += [jax.device_put(z, dev) for z in zero_outs]
        fn = jax.jit(make_body(), donate_argnums=donate, keep_unused=True)
        futures.append((fn(*args), out_names))

    results = []
    for out_arrs, out_names in futures:
        results.append({nm: np.asarray(a) for nm, a in zip(out_names, out_arrs)})
    return results


def time_launches(jobs, iters=8):
    """Re-execute prebuilt programs with device-resident inputs and return the
    best wall time per concurrent batch (seconds). No donation, outputs unused."""
    import time as _time

    import jax
    from concourse.bass2jax import _bass_exec_p, install_neuronx_cc_hook

    install_neuronx_cc_hook()
    devices = jax.devices()
    fns = []
    for nc, in_map, di in jobs:
        in_names, out_names, out_avals, zero_outs = _introspect(nc)
        pname = nc.partition_id_tensor.name if nc.partition_id_tensor else None
        all_names = tuple(in_names + out_names + ([pname] if pname else []))

        def make_body(nc=nc, out_avals=tuple(out_avals), all_names=all_names, out_names=tuple(out_names), pname=pname):
            def _body(*args):
                operands = list(args)
                if pname:
                    from concourse.bass2jax import partition_id_tensor

                    operands.append(partition_id_tensor())
                return tuple(
                    _bass_exec_p.bind(
                        *operands,
                        out_avals=out_avals,
                        in_names=all_names,
                        out_names=out_names,
                        lowering_input_output_aliases=(),
                        sim_require_finite=False,
                        sim_require_nnan=False,
                        nc=nc,
                    )
                )

            return _body

        dev = devices[di]
        args = [jax.device_put(np.ascontiguousarray(in_map[nm]), dev) for nm in in_names]
        args += [jax.device_put(z, dev) for z in zero_outs]
        fns.append((jax.jit(make_body(), keep_unused=True), args))

    outs = [fn(*a) for fn, a in fns]
    jax.block_until_ready(outs)
    best = float("inf")
    for _ in range(iters):
        t0 = _time.perf_counter()
        outs = [fn(*a) for fn, a in fns]
        jax.block_until_ready(outs)
        best = min(best, _time.perf_counter() - t0)
    return best


# ----------------------------------------------------------------------------
# Top level
# ----------------------------------------------------------------------------
_CACHE = {}
LAST_HW_TIME_NS = None


def kernel(x, edge_index, batch, W1, b1, W2, b2, Wg, bg, Wf, bf, Wo, bo):
    x = np.asarray(x, np.float32)
    n = x.shape[0]
    npad = -(-n // 2048) * 2048
    chunk = npad // N_CHUNKS
    assert chunk <= 32768

    key = (hash(np.asarray(edge_index).tobytes()), hash(np.asarray(batch).tobytes()))
    if key in _CACHE:
        metas, dinv, ncs1, ncs2, nc3 = _CACHE[key]
    else:
        metas, dinv = preprocess(edge_index, batch, npad, chunk)
        ncs1 = [build_launch1(m, chunk, npad) for m in metas]
        ncs2 = [build_launch2(m, chunk, npad) for m in metas]
        nc3 = build_launch3()
        _CACHE[key] = (metas, dinv, ncs1, ncs2, nc3)
        # Device-occupancy cost-model estimate of the HW execution time
        # (sum over the three launch phases of the slowest core's span).
        try:
            from concourse.timeline_sim import TimelineSim

            def _span(nc):
                ts = TimelineSim(nc, trace=False)
                ts.simulate()
                return ts.time

            global LAST_HW_TIME_NS
            LAST_HW_TIME_NS = int(
                max(_span(p) for p in ncs1)
                + max(_span(p) for p in ncs2)
                + _span(nc3)
            )
        except Exception:
            pass

    x_pad = np.zeros((npad, RP), np.float32)
    x_pad[:n, :F_IN] = x
    iota = _iota_const()

    # ---- launch 1: layer-1 aggregation + dense chain -> p (node sharded)
    jobs = []
    for c, m in enumerate(metas):
        in_map = {
            "x_pad": x_pad,
            "idx": m.idx_dram,
            "meta": m.meta_dram,
            "iota": iota,
            "W1": np.asarray(W1, np.float32),
            "W2": np.asarray(W2, np.float32),
            "b1": np.asarray(b1, np.float32).reshape(F_H, 1),
        }
        jobs.append((ncs1[c], in_map, c))
    res1 = run_programs(jobs)

    # ---- host relay: assemble the full gather table p_pad
    p_pad = np.zeros((npad, RP), np.float32)
    for c, m in enumerate(metas):
        pt = res1[c]["pT"]  # [F_P, n_super*SUPER]
        p_pad[m.ns : m.ne, :F_P] = pt[:, : m.n_slots].T

    # ---- launch 2: layer-2 aggregation + relu + per-graph max partials
    jobs = []
    for c, m in enumerate(metas):
        in_map = {
            "p_pad": p_pad,
            "idx": m.idx_dram,
            "meta": m.meta_dram,
            "iota": iota,
            "b2": np.asarray(b2, np.float32).reshape(F_P, 1),
        }
        jobs.append((ncs2[c], in_map, c))
    res2 = run_programs(jobs)

    # ---- combine pooled partials (tiny all-reduce, done host side)
    pooled = np.maximum.reduce([r["pooled"] for r in res2])  # [F_P, 256]

    # ---- launch 3: MLP head on one core
    in_map = {
        "pooled": pooled,
        "Wg": np.asarray(Wg, np.float32),
        "bg": np.asarray(bg, np.float32).reshape(64, 1),
        "Wf": np.asarray(Wf, np.float32),
        "bf": np.asarray(bf, np.float32).reshape(32, 1),
        "Wo": np.asarray(Wo, np.float32),
        "bo": np.asarray(bo, np.float32).reshape(1, 1),
    }
    res3 = run_programs([(nc3, in_map, 0)])
    return np.ascontiguousarray(res3[0]["out"].T)  # [256, 1]
